# revision 1
# baseline (speedup 1.0000x reference)
"""AttentionBlock (GroupNorm + single-head full attention + residual) on 8 TRN2 cores.

Data-parallel: batch B=8, one sample per NeuronCore. Per core:
  x [256, 4096] f32 -> groupnorm -> h (bf16)
  Algebraic folding (host-precomputed weight products):
    S[q,k] = q.k = sum_c h[c,q]*G2[c,k] + w[k] + c0
       G2 = M h + v,  M = Wq^T Wk, v = Wq^T b_k,  w[k] = (Wk^T b_q).h_k, c0 = b_q.b_k
    out_pre[q,co] = sum_k P[k,q]*VV[co,k],  VV = (Wo Wv) h + Wo b_v   (proj_out folded)
  softmax without max-subtraction (scores are O(+-3)); the softmax denominator
  rides as a ones-column of VV; the w[k]+c0 score bias rides as an extra VV
  projection column and enters via the Exp activation's per-partition bias.
  P^T layout [k, q] is produced directly by the S^T matmul, so no transpose of
  the 4096x4096 attention matrix is ever needed; only the final [4096, 256]
  attention output is transposed back to [c, n] via TensorE.
"""

import numpy as np

import concourse.bacc as bacc
import concourse.bass as bass
import concourse.tile as tile
from concourse import mybir
from concourse.bass_utils import run_bass_kernel_spmd

F32 = mybir.dt.float32
BF16 = mybir.dt.bfloat16
AF = mybir.ActivationFunctionType

C = 256          # channels
N = 4096         # spatial (64*64)
P = 128          # partitions
CT = C // P      # channel tiles (2)
NG = 8           # groups
GS = C // NG     # group size (32)
EPS = 1e-5
QB = 512         # queries per block
NQB = N // QB    # 8
NKT = N // P     # 32 k-tiles
SCALE = 1.0 / np.sqrt(C)  # 1/16


def _group_masks():
    # g0[p, g] = 1 if channel p (ct=0) is in group g; g1 likewise for ct=1
    g0 = np.zeros((P, NG), np.float32)
    g1 = np.zeros((P, NG), np.float32)
    for p in range(P):
        g0[p, p // GS] = 1.0
        g1[p, 4 + p // GS] = 1.0
    return g0, g1


def build_nc():
    nc = bacc.Bacc("TRN2", target_bir_lowering=False)

    x_d = nc.dram_tensor("x", [C, N], F32, kind="ExternalInput")
    mt_d = nc.dram_tensor("mt", [C, C], F32, kind="ExternalInput")      # lhsT[c',c] = M[c,c']
    vb_d = nc.dram_tensor("vb", [C], F32, kind="ExternalInput")         # v = Wq^T b_k
    w2t_d = nc.dram_tensor("w2t", [C, 258], F32, kind="ExternalInput")  # [W2^T | 0 | u]
    w2row_d = nc.dram_tensor("w2row", [1, 258], F32, kind="ExternalInput")  # [b2, 1, c0]
    bo_d = nc.dram_tensor("bo", [C], F32, kind="ExternalInput")
    out_d = nc.dram_tensor("out", [C, N], F32, kind="ExternalOutput")

    g0_np, g1_np = _group_masks()
    g0_d = nc.inline_tensor(g0_np, name="g0c")
    g1_d = nc.inline_tensor(g1_np, name="g1c")
    gt0_d = nc.inline_tensor(np.ascontiguousarray(g0_np.T), name="gt0c")
    gt1_d = nc.inline_tensor(np.ascontiguousarray(g1_np.T), name="gt1c")
    eye_d = nc.inline_tensor(np.eye(P, dtype=np.float32), name="eyec")

    import contextlib
    with tile.TileContext(nc) as tc, contextlib.ExitStack() as ctx:
        cst = ctx.enter_context(tc.tile_pool(name="cst", bufs=1))
        big = ctx.enter_context(tc.tile_pool(name="big", bufs=1))
        expp = ctx.enter_context(tc.tile_pool(name="expp", bufs=4))
        anp = ctx.enter_context(tc.tile_pool(name="anp", bufs=8))
        outp = ctx.enter_context(tc.tile_pool(name="outp", bufs=2))
        sml = ctx.enter_context(tc.tile_pool(name="sml", bufs=2))
        ps_s = ctx.enter_context(tc.tile_pool(name="ps_s", bufs=2, space="PSUM"))
        ps_o = ctx.enter_context(tc.tile_pool(name="ps_o", bufs=4, space="PSUM"))
        ps_t = ctx.enter_context(tc.tile_pool(name="ps_t", bufs=2, space="PSUM"))

        # ---- const loads + bf16 conversion ----
        mt_sb = cst.tile([P, CT, C], F32, name="mt_sb")
        nc.sync.dma_start(out=mt_sb, in_=mt_d.rearrange("(t p) c -> p t c", p=P))
        mtb = cst.tile([P, CT, C], BF16, name="mtb")
        nc.vector.tensor_copy(out=mtb, in_=mt_sb)

        w2t_sb = cst.tile([P, CT, 258], F32, name="w2t_sb")
        nc.sync.dma_start(out=w2t_sb, in_=w2t_d.rearrange("(t p) j -> p t j", p=P))
        w2tb = cst.tile([P, CT, 258], BF16, name="w2tb")
        nc.vector.tensor_copy(out=w2tb, in_=w2t_sb)

        w2row_sb = cst.tile([1, 258], F32, name="w2row_sb")
        nc.sync.dma_start(out=w2row_sb, in_=w2row_d[:, :])
        w2rowb = cst.tile([1, 258], BF16, name="w2rowb")
        nc.vector.tensor_copy(out=w2rowb, in_=w2row_sb)

        vb_sb = cst.tile([P, CT], F32, name="vb_sb")
        nc.sync.dma_start(out=vb_sb, in_=vb_d.rearrange("(t p) -> p t", p=P))
        bo_sb = cst.tile([P, CT], F32, name="bo_sb")
        nc.sync.dma_start(out=bo_sb, in_=bo_d.rearrange("(t p) -> p t", p=P))

        eye_sb = cst.tile([P, P], F32, name="eye_sb")
        nc.sync.dma_start(out=eye_sb, in_=eye_d[:, :])
        eyeb = cst.tile([P, P], BF16, name="eyeb")
        nc.vector.tensor_copy(out=eyeb, in_=eye_sb)

        g0_sb = cst.tile([P, NG], F32, name="g0_sb")
        nc.sync.dma_start(out=g0_sb, in_=g0_d[:, :])
        g1_sb = cst.tile([P, NG], F32, name="g1_sb")
        nc.sync.dma_start(out=g1_sb, in_=g1_d[:, :])
        gt0_sb = cst.tile([NG, P], F32, name="gt0_sb")
        nc.sync.dma_start(out=gt0_sb, in_=gt0_d[:, :])
        gt1_sb = cst.tile([NG, P], F32, name="gt1_sb")
        nc.sync.dma_start(out=gt1_sb, in_=gt1_d[:, :])

        ones_sb = cst.tile([1, P], BF16, name="ones_sb")
        nc.vector.memset(ones_sb, 1.0)
        eps_sb = cst.tile([NG, 1], F32, name="eps_sb")
        nc.vector.memset(eps_sb, EPS)

        # ---- x load ----
        x_sb = big.tile([P, CT, N], F32, name="x_sb")
        x_r = x_d.rearrange("(t p) n -> p t n", p=P)
        for ct in range(CT):
            nc.sync.dma_start(out=x_sb[:, ct, :], in_=x_r[:, ct, :])

        # ---- groupnorm stats ----
        NSG = N // 512  # 8 bn_stats subgroups per channel
        stats = sml.tile([P, CT, NSG, 6], F32, name="stats")
        mv = sml.tile([P, CT, 2], F32, name="mv")
        for ct in range(CT):
            for sg in range(NSG):
                nc.vector.bn_stats(
                    out=stats[:, ct, sg, :], in_=x_sb[:, ct, sg * 512:(sg + 1) * 512]
                )
            nc.vector.bn_aggr(out=mv[:, ct, :], in_=stats[:, ct, :, :])
        # per-partition [mean, var, mean^2]
        st3 = sml.tile([P, CT, 3], F32, name="st3")
        for ct in range(CT):
            nc.vector.tensor_copy(out=st3[:, ct, 0:2], in_=mv[:, ct, :])
            nc.vector.tensor_mul(
                out=st3[:, ct, 2:3], in0=mv[:, ct, 0:1], in1=mv[:, ct, 0:1]
            )
        gps = ps_s.tile([NG, 3], F32, name="gps", tag="s")
        nc.tensor.matmul(gps, lhsT=g0_sb, rhs=st3[:, 0, :], start=True, stop=False)
        nc.tensor.matmul(gps, lhsT=g1_sb, rhs=st3[:, 1, :], start=False, stop=True)
        gsb = sml.tile([NG, 3], F32, name="gsb")
        nc.vector.tensor_copy(out=gsb, in_=gps)
        # group mean / rstd  (means and vars avg over the 32 partitions of a group)
        gmean = sml.tile([NG, 1], F32, name="gmean")
        nc.vector.tensor_scalar_mul(out=gmean, in0=gsb[:, 0:1], scalar1=1.0 / GS)
        gtmp = sml.tile([NG, 1], F32, name="gtmp")
        nc.vector.tensor_add(out=gtmp, in0=gsb[:, 1:2], in1=gsb[:, 2:3])
        nc.vector.tensor_scalar_mul(out=gtmp, in0=gtmp, scalar1=1.0 / GS)
        gmsq = sml.tile([NG, 1], F32, name="gmsq")
        nc.vector.tensor_mul(out=gmsq, in0=gmean, in1=gmean)
        gvar = sml.tile([NG, 1], F32, name="gvar")
        nc.vector.tensor_sub(out=gvar, in0=gtmp, in1=gmsq)
        gstd = sml.tile([NG, 1], F32, name="gstd")
        nc.scalar.activation(out=gstd, in_=gvar, func=AF.Sqrt, bias=eps_sb)
        grstd = sml.tile([NG, 1], F32, name="grstd")
        nc.vector.reciprocal(out=grstd, in_=gstd)
        gpar = sml.tile([NG, 2], F32, name="gpar")
        nc.vector.tensor_copy(out=gpar[:, 0:1], in_=gmean)
        nc.vector.tensor_copy(out=gpar[:, 1:2], in_=grstd)
        # broadcast group params to channels
        mr_sb = sml.tile([P, CT, 2], F32, name="mr_sb")
        for ct, gt in ((0, gt0_sb), (1, gt1_sb)):
            bps = ps_s.tile([P, 2], F32, name=f"bps{ct}", tag="s")
            nc.tensor.matmul(bps, lhsT=gt, rhs=gpar, start=True, stop=True)
            nc.vector.tensor_copy(out=mr_sb[:, ct, :], in_=bps)
        # h = (x - mean) * rstd  -> bf16
        hb = big.tile([P, CT, N], BF16, name="hb")
        for ct in range(CT):
            nc.vector.tensor_scalar(
                out=hb[:, ct, :],
                in0=x_sb[:, ct, :],
                scalar1=mr_sb[:, ct, 0:1],
                scalar2=mr_sb[:, ct, 1:2],
                op0=mybir.AluOpType.subtract,
                op1=mybir.AluOpType.mult,
            )

        # ---- G2 = M h (+v bias on copy) ----
        g2b = big.tile([P, CT, N], BF16, name="g2b")
        for kb in range(NQB):
            for ct in range(CT):
                g2ps = ps_s.tile([P, QB], F32, name=f"g2ps_{kb}_{ct}", tag="s")
                ks = slice(kb * QB, (kb + 1) * QB)
                nc.tensor.matmul(
                    g2ps, lhsT=mtb[:, 0, ct * P:(ct + 1) * P], rhs=hb[:, 0, ks],
                    start=True, stop=False,
                )
                nc.tensor.matmul(
                    g2ps, lhsT=mtb[:, 1, ct * P:(ct + 1) * P], rhs=hb[:, 1, ks],
                    start=False, stop=True,
                )
                nc.vector.tensor_scalar_add(
                    out=g2b[:, ct, ks], in0=g2ps, scalar1=vb_sb[:, ct:ct + 1]
                )

        # ---- VV projection: [k, 258] per k-tile; col 256 = 1, col 257 = w[k]+c0 ----
        vvb = big.tile([P, NKT, 257], BF16, name="vvb")
        ebias = big.tile([P, NKT], F32, name="ebias")
        for kt in range(NKT):
            vps = ps_o.tile([P, 258], F32, name=f"vps_{kt}", tag="o")
            ks = slice(kt * P, (kt + 1) * P)
            nc.tensor.matmul(vps, lhsT=hb[:, 0, ks], rhs=w2tb[:, 0, :],
                             start=True, stop=False)
            nc.tensor.matmul(vps, lhsT=hb[:, 1, ks], rhs=w2tb[:, 1, :],
                             start=False, stop=False)
            nc.tensor.matmul(vps, lhsT=ones_sb, rhs=w2rowb, start=False, stop=True)
            nc.vector.tensor_copy(out=vvb[:, kt, :], in_=vps[:, 0:257])
            nc.vector.tensor_scalar_mul(
                out=ebias[:, kt:kt + 1], in0=vps[:, 257:258], scalar1=float(SCALE)
            )

        # ---- attention, per 512-query block ----
        def emit_epilogue(o_tiles, qb):
            tps = [
                ps_t.tile([P, QB], BF16, name=f"tps{ct}_{qb}", tag="t")
                for ct in range(CT)
            ]
            for qs in range(4):
                recip = sml.tile([P, 1], F32, name=f"recip_{qb}_{qs}", tag="recip")
                nc.vector.reciprocal(out=recip, in_=o_tiles[qs][:, 256:257])
                attn = anp.tile([P, C], BF16, name=f"attn_{qb}_{qs}", tag="attn")
                nc.vector.tensor_scalar_mul(
                    out=attn, in0=o_tiles[qs][:, 0:256], scalar1=recip
                )
                for ct in range(CT):
                    nc.tensor.transpose(
                        tps[ct][:, qs * P:(qs + 1) * P],
                        attn[:, ct * P:(ct + 1) * P],
                        eyeb,
                    )
            outt = outp.tile([P, CT, QB], F32, name=f"outt_{qb}", tag="outt")
            qs_ = slice(qb * QB, (qb + 1) * QB)
            for ct in range(CT):
                nc.vector.tensor_scalar_add(
                    out=outt[:, ct, :], in0=tps[ct], scalar1=bo_sb[:, ct:ct + 1]
                )
                nc.vector.tensor_add(
                    out=outt[:, ct, :], in0=outt[:, ct, :], in1=x_sb[:, ct, qs_]
                )
            out_r = out_d.rearrange("(t p) n -> p t n", p=P)
            nc.gpsimd.dma_start(out=out_r[:, :, qs_], in_=outt)

        prev_o = None
        prev_qb = None
        for qb in range(NQB):
            qs_ = slice(qb * QB, (qb + 1) * QB)
            o_tiles = [
                ps_o.tile([P, 258], F32, name=f"ops_{qb}_{qs}", tag="o")
                for qs in range(4)
            ]
            s_tiles = {}

            def emit_s(kt, qb=qb, qs_=qs_, s_tiles=s_tiles):
                sp = ps_s.tile([P, QB], F32, name=f"sps_{qb}_{kt}", tag="s")
                ks = slice(kt * P, (kt + 1) * P)
                nc.tensor.matmul(sp, lhsT=g2b[:, 0, ks], rhs=hb[:, 0, qs_],
                                 start=True, stop=False)
                nc.tensor.matmul(sp, lhsT=g2b[:, 1, ks], rhs=hb[:, 1, qs_],
                                 start=False, stop=True)
                s_tiles[kt] = sp

            emit_s(0)
            # previous block's epilogue lands between this block's first
            # S^T matmuls so the PE never waits on the ACT/DVE epilogue ops
            if prev_o is not None:
                emit_epilogue(prev_o, prev_qb)
            for kt in range(NKT):
                if kt + 1 < NKT:
                    emit_s(kt + 1)
                e = expp.tile([P, QB], BF16, name=f"e_{qb}_{kt}", tag="e")
                nc.scalar.activation(
                    out=e, in_=s_tiles.pop(kt), func=AF.Exp, scale=float(SCALE),
                    bias=ebias[:, kt:kt + 1],
                )
                for qs in range(4):
                    nc.tensor.matmul(
                        o_tiles[qs][:, 0:257],
                        lhsT=e[:, qs * P:(qs + 1) * P],
                        rhs=vvb[:, kt, :],
                        start=(kt == 0),
                        stop=(kt == NKT - 1),
                    )
            prev_o = o_tiles
            prev_qb = qb
        emit_epilogue(prev_o, prev_qb)

    nc.compile()
    return nc


_NC = None


def _get_nc():
    global _NC
    if _NC is None:
        _NC = build_nc()
    return _NC


def kernel(x, w_q, b_q, w_k, b_k, w_v, b_v, w_o, b_o):
    x = np.ascontiguousarray(np.asarray(x, np.float32))
    B = x.shape[0]
    wq = np.asarray(w_q, np.float32)
    wk = np.asarray(w_k, np.float32)
    wv = np.asarray(w_v, np.float32)
    wo = np.asarray(w_o, np.float32)
    bq = np.asarray(b_q, np.float32)
    bk = np.asarray(b_k, np.float32)
    bv = np.asarray(b_v, np.float32)
    bo = np.asarray(b_o, np.float32)

    mt = np.ascontiguousarray((wk.T @ wq).astype(np.float32))   # lhsT[c',c] = M[c,c']
    v = (wq.T @ bk).astype(np.float32)                          # score term h_q . v
    u = (wk.T @ bq).astype(np.float32)                          # score term u . h_k
    c0 = float(bq @ bk)
    w2 = (wo @ wv).astype(np.float32)
    b2 = (wo @ bv).astype(np.float32)
    w2t = np.zeros((C, 258), np.float32)
    w2t[:, :256] = w2.T
    w2t[:, 257] = u
    w2row = np.zeros((1, 258), np.float32)
    w2row[0, :256] = b2
    w2row[0, 256] = 1.0
    w2row[0, 257] = c0

    xr = x.reshape(B, C, N)
    shared = {"mt": mt, "vb": v, "w2t": w2t, "w2row": w2row, "bo": bo}
    in_maps = [{"x": np.ascontiguousarray(xr[i]), **shared} for i in range(B)]

    nc = _get_nc()
    res = run_bass_kernel_spmd(nc, in_maps, core_ids=list(range(B)))
    global _LAST
    _LAST = res
    out = np.stack([res.results[i]["out"] for i in range(B)], axis=0)
    return out.reshape(x.shape).astype(np.float32)


_LAST = None



# revision 5
# speedup vs baseline: 1.4233x; 1.4233x over previous
"""AttentionBlock (GroupNorm + single-head full attention + residual) on 8 TRN2 cores.

Data-parallel: batch B=8, one sample per NeuronCore. Per core:
  x [256, 4096] f32 -> groupnorm -> h (fp8 e4m3)
  Algebraic folding (host-precomputed weight products):
    S[q,k] = q.k = sum_c h[c,q]*G2[c,k] + w[k] + c0
       G2 = M h + v,  M = Wq^T Wk, v = Wq^T b_k,  w[k] = (Wk^T b_q).h_k, c0 = b_q.b_k
    out_pre[q,co] = sum_k P[k,q]*VV[co,k],  VV = (Wo Wv) h + Wo b_v   (proj_out folded)
  All heavy matmuls run in fp8 e4m3 with MatmulPerfMode.DoubleRow (contraction
  over 2 k-subtiles per instruction, 2x PE throughput).  The per-k score bias
  w[k]+c0 is folded multiplicatively into VV (f[k] = exp(SCALE*(w[k]+c0)),
  sum_k e*f*vv == sum_k (e*f)*vv), which makes the softmax-exp bias a constant
  (-SHIFT) so each ACT exp instruction can span two PSUM banks (1024 wide).
  The softmax denominator rides as a ones-column of VV (scaled by f).  P^T
  layout [k, q] comes straight out of the S^T matmul so the 4096x4096 attention
  matrix is never transposed; only the final [4096, 256] attention output is
  transposed back to [c, n] via TensorE.
"""

import numpy as np
import ml_dtypes

import concourse.bacc as bacc
import concourse.bass as bass
import concourse.tile as tile
from concourse import mybir
from concourse.bass_utils import run_bass_kernel_spmd

F32 = mybir.dt.float32
BF16 = mybir.dt.bfloat16
F8 = mybir.dt.float8e4
AF = mybir.ActivationFunctionType
DR = mybir.MatmulPerfMode.DoubleRow
ALU = mybir.AluOpType
F8NP = ml_dtypes.float8_e4m3fn

C = 256          # channels
N = 4096         # spatial (64*64)
P = 128          # partitions
CT = C // P      # channel tiles (2)
NG = 8           # groups
GS = C // NG     # group size (32)
EPS = 1e-5
QB = 512         # queries per block
NQB = N // QB    # 8
NKT = N // P     # 32 k-tiles
NPR = NKT // 2   # 16 k-tile pairs
SCALE = 1.0 / np.sqrt(C)  # 1/16
SHIFT = 3.0      # global exp shift (softmax-invariant), keeps fp8 e in range


def _group_masks():
    g0 = np.zeros((P, NG), np.float32)
    g1 = np.zeros((P, NG), np.float32)
    for p in range(P):
        g0[p, p // GS] = 1.0
        g1[p, 4 + p // GS] = 1.0
    return g0, g1


def build_nc():
    nc = bacc.Bacc("TRN2", target_bir_lowering=False)

    x_d = nc.dram_tensor("x", [C, N], F32, kind="ExternalInput")
    mt8_d = nc.dram_tensor("mt8", [P, CT, C], F8, kind="ExternalInput")
    vb_d = nc.dram_tensor("vb", [P, CT], F32, kind="ExternalInput")
    w2t8_d = nc.dram_tensor("w2t8", [P, CT, 258], F8, kind="ExternalInput")
    w2row_d = nc.dram_tensor("w2row", [1, 258], F32, kind="ExternalInput")
    bo_d = nc.dram_tensor("bo", [P, CT], F32, kind="ExternalInput")
    out_d = nc.dram_tensor("out", [C, N], F32, kind="ExternalOutput")

    g0_np, g1_np = _group_masks()
    g0_d = nc.inline_tensor(g0_np, name="g0c")
    g1_d = nc.inline_tensor(g1_np, name="g1c")
    gt0_d = nc.inline_tensor(np.ascontiguousarray(g0_np.T), name="gt0c")
    gt1_d = nc.inline_tensor(np.ascontiguousarray(g1_np.T), name="gt1c")
    eye_d = nc.inline_tensor(np.eye(P, dtype=np.float32), name="eyec")

    import contextlib
    with tile.TileContext(nc) as tc, contextlib.ExitStack() as ctx:
        cst = ctx.enter_context(tc.tile_pool(name="cst", bufs=1))
        big = ctx.enter_context(tc.tile_pool(name="big", bufs=1))
        e4p = ctx.enter_context(tc.tile_pool(name="e4p", bufs=2))
        anp = ctx.enter_context(tc.tile_pool(name="anp", bufs=4))
        outp = ctx.enter_context(tc.tile_pool(name="outp", bufs=2))
        sml = ctx.enter_context(tc.tile_pool(name="sml", bufs=2))
        ps_s = ctx.enter_context(tc.tile_pool(name="ps_s", bufs=2, space="PSUM"))
        ps_o = ctx.enter_context(tc.tile_pool(name="ps_o", bufs=2, space="PSUM"))
        ps_t = ctx.enter_context(tc.tile_pool(name="ps_t", bufs=2, space="PSUM"))

        # ---- const loads ----
        mt8_sb = cst.tile([P, CT, C], F8, name="mt8_sb")
        nc.sync.dma_start(out=mt8_sb, in_=mt8_d[:, :, :])
        w2t8_sb = cst.tile([P, CT, 258], F8, name="w2t8_sb")
        nc.sync.dma_start(out=w2t8_sb, in_=w2t8_d[:, :, :])
        w2row_sb = cst.tile([1, 258], F32, name="w2row_sb")
        nc.sync.dma_start(out=w2row_sb, in_=w2row_d[:, :])
        w2rowb = cst.tile([1, 258], BF16, name="w2rowb")
        nc.vector.tensor_copy(out=w2rowb, in_=w2row_sb)
        vb_sb = cst.tile([P, CT], F32, name="vb_sb")
        nc.sync.dma_start(out=vb_sb, in_=vb_d[:, :])
        bo_sb = cst.tile([P, CT], F32, name="bo_sb")
        nc.sync.dma_start(out=bo_sb, in_=bo_d[:, :])

        eye_sb = cst.tile([P, P], F32, name="eye_sb")
        nc.sync.dma_start(out=eye_sb, in_=eye_d[:, :])
        eyeb = cst.tile([P, P], BF16, name="eyeb")
        nc.vector.tensor_copy(out=eyeb, in_=eye_sb)

        g0_sb = cst.tile([P, NG], F32, name="g0_sb")
        nc.sync.dma_start(out=g0_sb, in_=g0_d[:, :])
        g1_sb = cst.tile([P, NG], F32, name="g1_sb")
        nc.sync.dma_start(out=g1_sb, in_=g1_d[:, :])
        gt0_sb = cst.tile([NG, P], F32, name="gt0_sb")
        nc.sync.dma_start(out=gt0_sb, in_=gt0_d[:, :])
        gt1_sb = cst.tile([NG, P], F32, name="gt1_sb")
        nc.sync.dma_start(out=gt1_sb, in_=gt1_d[:, :])

        ones_sb = cst.tile([1, P], BF16, name="ones_sb")
        nc.vector.memset(ones_sb, 1.0)
        eps_sb = cst.tile([NG, 1], F32, name="eps_sb")
        nc.vector.memset(eps_sb, EPS)
        nshift = cst.tile([P, 1], F32, name="nshift")
        nc.vector.memset(nshift, -SHIFT)
        zbias = cst.tile([P, 1], F32, name="zbias")
        nc.vector.memset(zbias, 0.0)

        # ---- x load ----
        x_sb = big.tile([P, CT, N], F32, name="x_sb")
        x_r = x_d.rearrange("(t p) n -> p t n", p=P)
        for ct in range(CT):
            nc.sync.dma_start(out=x_sb[:, ct, :], in_=x_r[:, ct, :])

        # ---- groupnorm stats ----
        NSG = N // 512
        stats = sml.tile([P, CT, NSG, 6], F32, name="stats")
        mv = sml.tile([P, CT, 2], F32, name="mv")
        for ct in range(CT):
            for sg in range(NSG):
                nc.vector.bn_stats(
                    out=stats[:, ct, sg, :], in_=x_sb[:, ct, sg * 512:(sg + 1) * 512]
                )
            nc.vector.bn_aggr(out=mv[:, ct, :], in_=stats[:, ct, :, :])
        st3 = sml.tile([P, CT, 3], F32, name="st3")
        for ct in range(CT):
            nc.vector.tensor_copy(out=st3[:, ct, 0:2], in_=mv[:, ct, :])
            nc.vector.tensor_mul(
                out=st3[:, ct, 2:3], in0=mv[:, ct, 0:1], in1=mv[:, ct, 0:1]
            )
        gps = ps_s.tile([NG, 3], F32, name="gps", tag="s")
        nc.tensor.matmul(gps, lhsT=g0_sb, rhs=st3[:, 0, :], start=True, stop=False)
        nc.tensor.matmul(gps, lhsT=g1_sb, rhs=st3[:, 1, :], start=False, stop=True)
        gsb = sml.tile([NG, 3], F32, name="gsb")
        nc.vector.tensor_copy(out=gsb, in_=gps)
        gmean = sml.tile([NG, 1], F32, name="gmean")
        nc.vector.tensor_scalar_mul(out=gmean, in0=gsb[:, 0:1], scalar1=1.0 / GS)
        gtmp = sml.tile([NG, 1], F32, name="gtmp")
        nc.vector.tensor_add(out=gtmp, in0=gsb[:, 1:2], in1=gsb[:, 2:3])
        nc.vector.tensor_scalar_mul(out=gtmp, in0=gtmp, scalar1=1.0 / GS)
        gmsq = sml.tile([NG, 1], F32, name="gmsq")
        nc.vector.tensor_mul(out=gmsq, in0=gmean, in1=gmean)
        gvar = sml.tile([NG, 1], F32, name="gvar")
        nc.vector.tensor_sub(out=gvar, in0=gtmp, in1=gmsq)
        gstd = sml.tile([NG, 1], F32, name="gstd")
        nc.scalar.activation(out=gstd, in_=gvar, func=AF.Sqrt, bias=eps_sb)
        grstd = sml.tile([NG, 1], F32, name="grstd")
        nc.vector.reciprocal(out=grstd, in_=gstd)
        gpar = sml.tile([NG, 2], F32, name="gpar")
        nc.vector.tensor_copy(out=gpar[:, 0:1], in_=gmean)
        nc.vector.tensor_copy(out=gpar[:, 1:2], in_=grstd)
        mr_sb = sml.tile([P, CT, 2], F32, name="mr_sb")
        for ct, gt in ((0, gt0_sb), (1, gt1_sb)):
            bps = ps_s.tile([P, 2], F32, name=f"bps{ct}", tag="s")
            nc.tensor.matmul(bps, lhsT=gt, rhs=gpar, start=True, stop=True)
            nc.vector.tensor_copy(out=mr_sb[:, ct, :], in_=bps)
        # h = (x - mean) * rstd -> fp8
        hf8 = big.tile([P, CT, N], F8, name="hf8")
        for ct in range(CT):
            nc.vector.tensor_scalar(
                out=hf8[:, ct, :],
                in0=x_sb[:, ct, :],
                scalar1=mr_sb[:, ct, 0:1],
                scalar2=mr_sb[:, ct, 1:2],
                op0=ALU.subtract,
                op1=ALU.mult,
            )

        # ---- G2 = (16M) h / 16 + v   (DoubleRow fp8) ----
        g2f8 = big.tile([P, CT, N], F8, name="g2f8")
        for kb in range(NQB):
            ks = slice(kb * QB, (kb + 1) * QB)
            for ct in range(CT):
                g2ps = ps_s.tile([P, QB], F32, name=f"g2ps_{kb}_{ct}", tag="s")
                nc.tensor.matmul(
                    g2ps, lhsT=mt8_sb[:, :, ct * P:(ct + 1) * P],
                    rhs=hf8[:, :, ks], start=True, stop=True, perf_mode=DR,
                )
                nc.vector.tensor_scalar(
                    out=g2f8[:, ct, ks], in0=g2ps,
                    scalar1=1.0 / 16.0, scalar2=vb_sb[:, ct:ct + 1],
                    op0=ALU.mult, op1=ALU.add,
                )

        # ---- VV projection + f = exp(SCALE*(w+c0)) folding ----
        # vv8[k, 0:256] = 16*VV[k, co]*f[k]; vv8[k, 256] = f[k]
        vv8 = big.tile([P, NKT, 257], F8, name="vv8")
        fz = big.tile([P, NKT], F32, name="fz")
        fex = big.tile([P, NKT], F32, name="fex")

        def emit_vv_mm(kt):
            vps = ps_o.tile([P, 258], F32, name=f"vps_{kt}", tag="o")
            ks = slice(kt * P, (kt + 1) * P)
            nc.tensor.matmul(vps, lhsT=hf8[:, :, ks], rhs=w2t8_sb,
                             start=True, stop=False, perf_mode=DR)
            nc.tensor.matmul(vps, lhsT=ones_sb, rhs=w2rowb,
                             start=False, stop=True)
            return vps

        def emit_vv_pair(i):
            kt0, kt1 = 2 * i, 2 * i + 1
            vps0 = emit_vv_mm(kt0)
            vps1 = emit_vv_mm(kt1)
            for kt, vps in ((kt0, vps0), (kt1, vps1)):
                nc.vector.tensor_scalar_mul(
                    out=fz[:, kt:kt + 1], in0=vps[:, 257:258],
                    scalar1=float(SCALE / 16.0),
                )
            nc.scalar.activation(
                out=fex[:, kt0:kt1 + 1], in_=fz[:, kt0:kt1 + 1],
                func=AF.Exp, bias=zbias,
            )
            for kt, vps in ((kt0, vps0), (kt1, vps1)):
                nc.vector.tensor_scalar_mul(
                    out=vv8[:, kt, :], in0=vps[:, 0:257], scalar1=fex[:, kt:kt + 1]
                )

        # ---- attention ----
        e4_tiles = {}
        o_cur = {}

        def emit_s_pair(qb, t):
            qs_ = slice(qb * QB, (qb + 1) * QB)
            sp = ps_s.tile([P, 2, QB], F32, name=f"sps_{qb}_{t}", tag="s")
            for j in (0, 1):
                kt = 2 * t + j
                nc.tensor.matmul(
                    sp[:, j, :], lhsT=g2f8[:, :, kt * P:(kt + 1) * P],
                    rhs=hf8[:, :, qs_], start=True, stop=True, perf_mode=DR,
                )
            nc.scalar.activation(
                out=e4_tiles[qb][:, 2 * t:2 * t + 2, :], in_=sp,
                func=AF.Exp, scale=float(SCALE), bias=nshift,
            )

        def emit_pv(qb, qs, t):
            if t == 0:
                o_cur[qs] = ps_o.tile([P, 258], F32, name=f"ops_{qb}_{qs}", tag="o")
            nc.tensor.matmul(
                o_cur[qs][:, 0:257],
                lhsT=e4_tiles[qb][:, 2 * t:2 * t + 2, qs * P:(qs + 1) * P],
                rhs=vv8[:, 2 * t:2 * t + 2, :],
                start=(t == 0), stop=(t == NPR - 1), perf_mode=DR,
            )

        def emit_qs_epilogue(qb, qs, tps):
            o = o_cur[qs]
            recip = sml.tile([P, 1], F32, name=f"rc_{qb}_{qs}", tag="recip")
            nc.vector.reciprocal(out=recip, in_=o[:, 256:257])
            recip2 = sml.tile([P, 1], F32, name=f"rc2_{qb}_{qs}", tag="recip2")
            nc.vector.tensor_scalar_mul(out=recip2, in0=recip, scalar1=1.0 / 16.0)
            attn = anp.tile([P, C], BF16, name=f"attn_{qb}_{qs}", tag="attn")
            nc.vector.tensor_scalar_mul(out=attn, in0=o[:, 0:256], scalar1=recip2)
            for ct in range(CT):
                nc.tensor.transpose(
                    tps[:, ct, qs * P:(qs + 1) * P],
                    attn[:, ct * P:(ct + 1) * P],
                    eyeb,
                )

        def emit_qb_out(qb, tps):
            outt = outp.tile([P, CT, QB], F32, name=f"outt_{qb}", tag="outt")
            qs_ = slice(qb * QB, (qb + 1) * QB)
            for ct in range(CT):
                nc.vector.tensor_scalar_add(
                    out=outt[:, ct, :], in0=tps[:, ct, :], scalar1=bo_sb[:, ct:ct + 1]
                )
                nc.vector.tensor_add(
                    out=outt[:, ct, :], in0=outt[:, ct, :], in1=x_sb[:, ct, qs_]
                )
            out_r = out_d.rearrange("(t p) n -> p t n", p=P)
            nc.gpsimd.dma_start(out=out_r[:, :, qs_], in_=outt)

        # aux work interleaved into the S phase of each q-block:
        #   qb 0: the 32 VV matmul groups; qb >= 1: the 64 PV matmuls of qb-1.
        tps_cur = {}

        def aux_pv(qb_prev, i):  # i in 0..15 -> 4 PV matmuls per step
            for k in range(4):
                idx = 4 * i + k
                qs, t = divmod(idx, NPR)
                emit_pv(qb_prev, qs, t)
                if t == NPR - 1:
                    if qs == 0:
                        tps_cur[qb_prev] = ps_t.tile(
                            [P, CT, QB], BF16, name=f"tps_{qb_prev}", tag="t"
                        )
                    emit_qs_epilogue(qb_prev, qs, tps_cur[qb_prev])
                    if qs == 3:
                        emit_qb_out(qb_prev, tps_cur.pop(qb_prev))

        for qb in range(NQB):
            e4_tiles[qb] = e4p.tile([P, NKT, QB], F8, name=f"e4_{qb}", tag="e4")
            if qb >= 2:
                del e4_tiles[qb - 2]
            for t in range(NPR):
                emit_s_pair(qb, t)
                if qb == 0:
                    emit_vv_pair(t)
                else:
                    aux_pv(qb - 1, t)
        for i in range(NPR):
            aux_pv(NQB - 1, i)

    nc.compile()
    return nc


_NC = None


def _get_nc():
    global _NC
    if _NC is None:
        _NC = build_nc()
    return _NC


def _host_prep(x, w_q, b_q, w_k, b_k, w_v, b_v, w_o, b_o):
    x = np.ascontiguousarray(np.asarray(x, np.float32))
    B = x.shape[0]
    wq = np.asarray(w_q, np.float32)
    wk = np.asarray(w_k, np.float32)
    wv = np.asarray(w_v, np.float32)
    wo = np.asarray(w_o, np.float32)
    bq = np.asarray(b_q, np.float32)
    bk = np.asarray(b_k, np.float32)
    bv = np.asarray(b_v, np.float32)
    bo = np.asarray(b_o, np.float32)

    def to_pt(a):  # [C, ...] -> [P, CT, ...]
        return np.ascontiguousarray(
            a.reshape(CT, P, *a.shape[1:]).transpose(1, 0, *range(2, a.ndim + 1))
        )

    mt = (wk.T @ wq).astype(np.float32)       # lhsT[c, c'] = M[c', c]
    mt8 = to_pt((16.0 * mt).astype(F8NP))
    v = (wq.T @ bk).astype(np.float32)
    u = (wk.T @ bq).astype(np.float32)
    c0 = float(bq @ bk)
    w2 = (wo @ wv).astype(np.float32)
    b2 = (wo @ bv).astype(np.float32)
    w2t = np.zeros((C, 258), np.float32)
    w2t[:, :256] = 16.0 * w2.T
    w2t[:, 257] = 16.0 * u
    w2t8 = to_pt(w2t.astype(F8NP))
    w2row = np.zeros((1, 258), np.float32)
    w2row[0, :256] = 16.0 * b2
    w2row[0, 256] = 1.0
    w2row[0, 257] = 16.0 * c0

    xr = x.reshape(B, C, N)
    shared = {
        "mt8": mt8, "vb": to_pt(v), "w2t8": w2t8, "w2row": w2row, "bo": to_pt(bo),
    }
    in_maps = [{"x": np.ascontiguousarray(xr[i]), **shared} for i in range(B)]
    return x, in_maps


def kernel(x, w_q, b_q, w_k, b_k, w_v, b_v, w_o, b_o):
    x, in_maps = _host_prep(x, w_q, b_q, w_k, b_k, w_v, b_v, w_o, b_o)
    B = x.shape[0]
    nc = _get_nc()
    res = run_bass_kernel_spmd(nc, in_maps, core_ids=list(range(B)))
    global _LAST
    _LAST = res
    out = np.stack([res.results[i]["out"] for i in range(B)], axis=0)
    return out.reshape(x.shape).astype(np.float32)


_LAST = None
